# revision 1
# baseline (speedup 1.0000x reference)
"""GATv2 layer (heads=1) + post leaky-relu + batchnorm on 8 Trainium2 cores.

Strategy (dst-sharded edge parallelism):
  - Host sorts edges by dst node. Core c owns dst nodes [c*npc, (c+1)*npc).
  - Each core's dst nodes are grouped in blocks of 111; each block's edge
    list is padded to a uniform number of 128-edge chunks (SPMD static loops).
  - Node transforms xl = x@W_l (plus the att-scaled variant used by the
    logits) are computed on every core (replicated) into a DRAM table, which
    is then row-gathered per edge chunk with dma_gather.
  - Per chunk, one fused matmul computes the edge-attr projection AND the
    xr[dst] broadcast AND the linear part of the attention logit:
        lhsT = [onehotT(111) ; edge_attr.T(16) ; a_l_row(1)]  (K=128)
        rhs  = [xr'_blk ; W_e' ; e_127] with col 128 carrying att-dot terms
  - leaky_relu(m)@att is computed exactly as 0.2*A + 0.8*(r1 - r2) where
    A = att.msg (linear, from psum col 128) and r1/r2 are Relu row-sums over
    att-positive / att-negative feature columns (features pre-permuted and
    pre-scaled by |att| on the host). Everything stays in the one ACT table
    that has both Relu and Exp (no activation-table thrash).
  - Softmax without max-subtraction (logits are in [-7, 6]; exp is safe in
    f32 and the segment max cancels exactly in alpha = p/denom).
  - Scatter-add via segment-indicator matmul: U[s,:] += onehot.T @ [p*xl | p]
    accumulated in PSUM over each block's chunks; out = U/D + bias, leaky.
  - BatchNorm (training-mode batch stats over all nodes) on host.
"""
import sys

if "/opt/trn_rl_repo" not in sys.path:
    sys.path.insert(0, "/opt/trn_rl_repo")

import numpy as np

NEG_SLOPE = 0.2
BN_EPS = 1e-5

P = 128
NCORES = 8
BLK = 111            # dst nodes per block (111 + 16 + 1 = 128 = fused matmul K)
F = 128              # feature dim
ED = 16              # edge-attr dim
NH = 4               # gather batches per block

# precision/perf flags (A/B on hardware)
SEG_F32R = False     # segment matmul in float32r with N padded to 256


class Plan:
    """Geometry + host-prepped per-core inputs for one problem size."""

    def __init__(self, x, edge_attr, edge_index, W_l, W_r, W_e, att, bias,
                 ncores=NCORES):
        x = np.ascontiguousarray(np.asarray(x, dtype=np.float32))
        edge_attr = np.ascontiguousarray(np.asarray(edge_attr, dtype=np.float32))
        W_l = np.asarray(W_l, dtype=np.float32)
        W_r = np.asarray(W_r, dtype=np.float32)
        W_e = np.asarray(W_e, dtype=np.float32)
        att = np.asarray(att, dtype=np.float32)
        bias = np.asarray(bias, dtype=np.float32)
        src = np.asarray(edge_index[0]).astype(np.int64)
        dst = np.asarray(edge_index[1]).astype(np.int64)

        n = x.shape[0]
        self.n = n
        self.ncores = ncores
        self.npc = -(-n // ncores)                  # dst nodes per core
        self.nblk = -(-self.npc // BLK)             # blocks per core
        self.nt = -(-n // P)                        # transform tiles
        self.npad = self.nt * P
        need = (ncores - 1) * self.npc + self.nblk * BLK
        while self.npad < need:
            self.nt += 1
            self.npad = self.nt * P
        assert self.npad < 32768, "dma_gather int16 indices"

        order = np.argsort(dst, kind="stable")
        src_s, dst_s, ea_s = src[order], dst[order], edge_attr[order]

        blk_lo = np.empty(ncores * self.nblk, dtype=np.int64)
        blk_hi = np.empty(ncores * self.nblk, dtype=np.int64)
        for c in range(ncores):
            for j in range(self.nblk):
                i = c * self.nblk + j
                lo_node = c * self.npc + j * BLK
                hi_node = min(lo_node + BLK, (c + 1) * self.npc)
                blk_lo[i] = np.searchsorted(dst_s, lo_node)
                blk_hi[i] = np.searchsorted(dst_s, hi_node)
        counts = blk_hi - blk_lo
        nch = max(NH, int(-(-counts.max() // P)))
        nch += (-nch) % NH                          # multiple of NH
        self.nch = nch
        self.epc = self.nblk * nch * P              # padded edges per core
        self.nchc = self.nblk * nch                 # chunks per core

        # feature permutation: att-positive first, scaled by |att|
        pos = att > 0
        pi = np.concatenate([np.nonzero(pos)[0], np.nonzero(~pos)[0]])
        self.ppos = int(pos.sum())
        aabs = (4.0 * np.abs(att[pi])).astype(np.float32)

        w_la = (W_l[:, pi] * aabs[None, :]).astype(np.float32)
        self.wcat = np.ascontiguousarray(
            np.concatenate([W_l, w_la], axis=1), dtype=np.float32)  # [F, 2F]
        self.wrx = np.ascontiguousarray(np.concatenate(
            [W_r[:, pi] * aabs[None, :], (W_r @ att)[:, None]], axis=1),
            dtype=np.float32)                                       # [F, F+1]
        wecx = np.zeros((ED + 1, F + 1), dtype=np.float32)
        wecx[:ED, :F] = W_e[:, pi] * aabs[None, :]
        wecx[:ED, F] = W_e @ att
        wecx[ED, F] = 1.0
        self.wecx = wecx                                            # [17, F+1]
        self.bias_bc = np.ascontiguousarray(
            np.tile(bias[None, :], (BLK, 1)))
        self.iota_r = np.ascontiguousarray(
            np.tile(np.arange(BLK, dtype=np.float32)[None, :], (P, 1)))
        self.iota_c = np.ascontiguousarray(
            np.arange(BLK, dtype=np.float32)[:, None])

        xt = np.zeros((F, self.npad), dtype=np.float32)
        xt[:, :n] = x.T
        self.xt = xt
        a_l = (x @ (W_l @ att)).astype(np.float32)                  # [n]

        self.cores = []
        for c in range(ncores):
            eatx = np.zeros((ED + 1, self.epc), dtype=np.float32)
            srcidx = np.zeros(self.epc, dtype=np.int16)
            dstrel = np.full(self.epc, 120.0, dtype=np.float32)
            for j in range(self.nblk):
                i = c * self.nblk + j
                lo, hi = blk_lo[i], blk_hi[i]
                m = hi - lo
                if m == 0:
                    continue
                base = j * nch * P
                assert m <= nch * P
                eatx[:ED, base:base + m] = ea_s[lo:hi].T
                eatx[ED, base:base + m] = a_l[src_s[lo:hi]]
                srcidx[base:base + m] = src_s[lo:hi]
                dstrel[base:base + m] = dst_s[lo:hi] - c * self.npc - j * BLK
            srcw = np.tile(srcidx.reshape(self.epc // 16, 16).T, (8, 1))
            self.cores.append(dict(
                eatx=np.ascontiguousarray(eatx),
                srcw=np.ascontiguousarray(srcw),
                dstc=np.ascontiguousarray(
                    dstrel.reshape(self.nchc, P).T),     # [P, nchc]
                dstr=np.ascontiguousarray(dstrel[None, :]),
                xtc=np.ascontiguousarray(
                    xt[:, c * self.npc: c * self.npc + self.nblk * BLK]),
            ))

    def in_maps(self):
        shared = dict(xt=self.xt, wcat=self.wcat, wrx=self.wrx,
                      wecx=self.wecx, biasr=self.bias_bc,
                      iotar=self.iota_r, iotac=self.iota_c)
        return [dict(shared, **c) for c in self.cores]


def build_program(plan, num_devices=None, nch_run=None, nblk_run=None):
    import concourse.bacc as bacc
    import concourse.mybir as mybir
    import concourse.tile as tile

    dt = mybir.dt
    f32 = dt.float32
    AF = mybir.ActivationFunctionType
    OP = mybir.AluOpType
    ts = lambda i, sz: slice(i * sz, (i + 1) * sz)

    nch, nblk, nt, npad = plan.nch, plan.nblk, plan.nt, plan.npad
    epc, ppos = plan.epc, plan.ppos
    nch_run = nch if nch_run is None else nch_run      # timing experiments
    nblk_run = nblk if nblk_run is None else nblk_run
    g = nch_run // NH                # chunks per gather batch
    GROW = 256                       # gather row: [xl(128) | xla(128)] f32

    nc = bacc.Bacc("TRN2", target_bir_lowering=False, debug=False,
                   num_devices=num_devices or plan.ncores)

    t_xt = nc.dram_tensor("xt", [F, npad], f32, kind="ExternalInput")
    t_xtc = nc.dram_tensor("xtc", [F, nblk * BLK], f32, kind="ExternalInput")
    t_wcat = nc.dram_tensor("wcat", [F, 2 * F], f32, kind="ExternalInput")
    t_wrx = nc.dram_tensor("wrx", [F, F + 1], f32, kind="ExternalInput")
    t_wecx = nc.dram_tensor("wecx", [ED + 1, F + 1], f32, kind="ExternalInput")
    t_biasr = nc.dram_tensor("biasr", [BLK, F], f32, kind="ExternalInput")
    t_iotar = nc.dram_tensor("iotar", [P, BLK], f32, kind="ExternalInput")
    t_iotac = nc.dram_tensor("iotac", [BLK, 1], f32, kind="ExternalInput")
    t_eatx = nc.dram_tensor("eatx", [ED + 1, epc], f32, kind="ExternalInput")
    t_srcw = nc.dram_tensor("srcw", [P, epc // 16], dt.int16, kind="ExternalInput")
    t_dstc = nc.dram_tensor("dstc", [P, plan.nchc], f32, kind="ExternalInput")
    t_dstr = nc.dram_tensor("dstr", [1, epc], f32, kind="ExternalInput")

    t_xlc = nc.dram_tensor("xlc", [npad, GROW], f32, kind="Internal")
    t_out = nc.dram_tensor("out", [nblk * BLK, F], f32, kind="ExternalOutput")

    with tile.TileContext(nc) as tc:
        with tc.tile_pool(name="resident", bufs=1) as rpool:

            # ---------- phase T: node transforms ----------
            wcat_sb = rpool.tile([F, 2 * F], f32, tag="wcat")
            nc.sync.dma_start(wcat_sb[:], t_wcat.ap())
            wrx_sb = rpool.tile([F, F + 1], f32, tag="wrx")
            nc.sync.dma_start(wrx_sb[:], t_wrx.ap())
            biasr_sb = rpool.tile([BLK, F], f32, tag="biasr")
            nc.sync.dma_start(biasr_sb[:], t_biasr.ap())
            dstc_sb = rpool.tile([P, plan.nchc], f32, tag="dstc")
            nc.sync.dma_start(dstc_sb[:], t_dstc.ap())
            srcw_sb = rpool.tile([P, epc // 16], dt.int16, tag="srcw")
            nc.sync.dma_start(srcw_sb[:], t_srcw.ap())
            iota_r = rpool.tile([P, BLK], f32, tag="iotar")
            nc.sync.dma_start(iota_r[:], t_iotar.ap())
            iota_c = rpool.tile([BLK, 1], f32, tag="iotac")
            nc.sync.dma_start(iota_c[:], t_iotac.ap())

            rhs_blk = [rpool.tile([P, F + 1], f32, tag=f"rhsblk{b}",
                                  name=f"rhsblk{b}")
                       for b in range(nblk)]

            with tc.tile_pool(name="xbig", bufs=1) as xbig, \
                 tc.tile_pool(name="xstage", bufs=3) as xstg, \
                 tc.tile_pool(name="xpsum", bufs=2, space="PSUM") as xpsum:
                xt_sb = xbig.tile([F, npad], f32, tag="xt")
                nc.sync.dma_start(xt_sb[:], t_xt.ap())
                xtc_sb = xbig.tile([F, nblk * BLK], f32, tag="xtc")
                nc.sync.dma_start(xtc_sb[:], t_xtc.ap())
                for t in range(nt):
                    ps = xpsum.tile([P, GROW], f32, tag="xps")
                    nc.tensor.matmul(ps[:], lhsT=xt_sb[:, ts(t, P)],
                                     rhs=wcat_sb[:], start=True, stop=True)
                    st = xstg.tile([P, GROW], f32, tag="xstage")
                    nc.vector.tensor_copy(st[:], ps[:])
                    nc.sync.dma_start(t_xlc.ap()[ts(t, P), :], st[:])
                for b in range(nblk):
                    ps2 = xpsum.tile([BLK, F + 1], f32, tag="xps2")
                    nc.tensor.matmul(ps2[:], lhsT=xtc_sb[:, ts(b, BLK)],
                                     rhs=wrx_sb[:], start=True, stop=True)
                    nc.vector.tensor_copy(rhs_blk[b][0:BLK, :], ps2[:])
                    nc.sync.dma_start(rhs_blk[b][BLK:P, :], t_wecx.ap())

            # ---------- phase E: edges ----------
            with tc.tile_pool(name="edges", bufs=2) as epool, \
                 tc.tile_pool(name="small", bufs=3) as spool, \
                 tc.tile_pool(name="chunk", bufs=8) as cpool, \
                 tc.tile_pool(name="mpsum", bufs=4, space="PSUM") as mpsum, \
                 tc.tile_pool(name="spsum", bufs=2, space="PSUM") as spsum, \
                 tc.tile_pool(name="upsum", bufs=2, space="PSUM") as upsum, \
                 tc.tile_pool(name="outp", bufs=2) as opool:
                useg = F + 1
                for b in range(nblk_run):
                    u_ps = upsum.tile([BLK, useg], f32, tag="useg")
                    for h in range(NH):
                        q0 = b * nch + h * g
                        e0 = q0 * P
                        xg = epool.tile([P, g, GROW], f32, tag="xg")
                        nc.gpsimd.dma_gather(
                            xg[:], t_xlc.ap(),
                            srcw_sb[:, e0 // 16:(e0 + g * P) // 16],
                            g * P, g * P, GROW,
                            single_packet=(g * P <= 512))
                        lst = epool.tile([P, g * P], f32, tag="lst")
                        nc.sync.dma_start(lst[BLK:P, :],
                                          t_eatx.ap()[:, e0:e0 + g * P])
                        dsr = spool.tile([1, g * P], f32, tag="dsr")
                        nc.sync.dma_start(dsr[:], t_dstr.ap()[:, e0:e0 + g * P])
                        dsb = epool.tile([BLK, g * P], f32, tag="dsb")
                        nc.gpsimd.partition_broadcast(dsb[:], dsr[:],
                                                      channels=BLK)
                        r1b = spool.tile([P, g], f32, tag="r1b")
                        r2b = spool.tile([P, g], f32, tag="r2b")
                        emb = spool.tile([P, g], f32, tag="emb")
                        pb = spool.tile([P, g], f32, tag="pb")
                        if ppos == 0:
                            nc.vector.memset(r1b[:], 0.0)
                        if ppos == F:
                            nc.vector.memset(r2b[:], 0.0)
                        rhs2s = epool.tile([P, g, F + 1], f32, tag="rhs2")
                        for k in range(g):
                            q = q0 + k
                            nc.vector.tensor_scalar(
                                lst[0:BLK, ts(k, P)], dsb[:, ts(k, P)],
                                iota_c[:], None, OP.is_equal)
                            m_ps = mpsum.tile([P, F + 1], f32, tag="mps")
                            nc.tensor.matmul(m_ps[:], lhsT=lst[:, ts(k, P)],
                                             rhs=rhs_blk[b][:],
                                             start=True, stop=True)
                            u = cpool.tile([P, F], f32, tag="u")
                            nc.vector.tensor_tensor(
                                u[:], m_ps[:, 0:F], xg[:, k, F:2 * F], OP.add)
                            scr = cpool.tile([P, F], f32, tag="scr")
                            p1 = ppos
                            p2 = F - ppos
                            if ppos > 0:
                                nc.scalar.activation(
                                    scr[:, 0:p1], u[:, 0:p1], AF.Relu,
                                    accum_out=r1b[:, k:k + 1])
                            if ppos < F:
                                nc.scalar.activation(
                                    scr[:, F - p2:F], u[:, F - p2:F], AF.Relu,
                                    accum_out=r2b[:, k:k + 1])
                            nc.vector.tensor_scalar(
                                emb[:, k:k + 1], m_ps[:, F:F + 1],
                                r1b[:, k:k + 1], r2b[:, k:k + 1],
                                OP.add, OP.subtract)
                        nc.scalar.activation(pb[:], emb[:], AF.Exp,
                                             scale=NEG_SLOPE)
                        nc.vector.tensor_copy(rhs2s[:, :, F], pb[:])
                        for k in range(g):
                            q = q0 + k
                            nc.vector.tensor_scalar(
                                rhs2s[:, k, 0:F], xg[:, k, 0:F],
                                pb[:, k:k + 1], None, OP.mult)
                            oh = cpool.tile([P, BLK], f32, tag="oh")
                            nc.vector.tensor_scalar(
                                oh[:], iota_r[:], dstc_sb[:, q:q + 1], None,
                                OP.is_equal)
                            nc.tensor.matmul(
                                u_ps[:], lhsT=oh[:], rhs=rhs2s[:, k, :],
                                start=(q == b * nch),
                                stop=(q == b * nch + nch_run - 1))
                    # block epilogue: out = leaky(U/D + bias)
                    dcol = opool.tile([BLK, 1], f32, tag="dcol")
                    nc.vector.reciprocal(dcol[:], u_ps[:, F:F + 1])
                    ob = opool.tile([BLK, F], f32, tag="ob")
                    nc.vector.tensor_scalar(ob[:], u_ps[:, 0:F], dcol[:],
                                            None, OP.mult)
                    nc.vector.tensor_tensor(
                        ob[:], ob[:], biasr_sb[:], OP.add)
                    ob2 = opool.tile([BLK, F], f32, tag="ob2")
                    nc.vector.tensor_scalar(ob2[:], ob[:], NEG_SLOPE, None,
                                            OP.mult)
                    nc.vector.tensor_tensor(ob2[:], ob2[:], ob[:], OP.max)
                    nc.sync.dma_start(t_out.ap()[ts(b, BLK), :], ob2[:])

    nc.compile()
    return nc


def run_plan(plan, nc=None, trace=False):
    from concourse import bass_utils
    if nc is None:
        nc = build_program(plan)
    return bass_utils.run_bass_kernel_spmd(
        nc, plan.in_maps(), core_ids=list(range(plan.ncores)), trace=trace)


def assemble(plan, results):
    """Concat per-core outputs, slice to real nodes, apply host batchnorm."""
    outs = []
    for c in range(plan.ncores):
        o = np.asarray(results[c]["out"])
        lo = c * plan.npc
        take = min(plan.npc, plan.n - lo)
        outs.append(o[:take])
    out = np.concatenate(outs, axis=0)
    mean = out.mean(axis=0)
    var = out.var(axis=0)
    return ((out - mean) / np.sqrt(var + BN_EPS)).astype(np.float32)


_CACHE = {}


def kernel(x, edge_attr, edge_index, W_l, W_r, W_e, att, bias,
           bn_weight, bn_bias):
    plan = Plan(x, edge_attr, edge_index, W_l, W_r, W_e, att, bias)
    key = (plan.n, plan.nch, plan.ppos)
    nc = _CACHE.get(key)
    if nc is None:
        nc = build_program(plan)
        _CACHE[key] = nc
    res = run_plan(plan, nc=nc)
    out = assemble(plan, res.results)
    bn_w = np.asarray(bn_weight, dtype=np.float32)
    bn_b = np.asarray(bn_bias, dtype=np.float32)
    return (out * bn_w[None, :] + bn_b[None, :]).astype(np.float32)



# revision 24
# speedup vs baseline: 83.1147x; 83.1147x over previous
"""GATv2 layer (heads=1) + post leaky-relu + batchnorm on 8 Trainium2 cores.

v2 strategy (dst-sharded edge parallelism, feature-major message pipeline):
  - Host sorts edges by dst. Core c owns dst nodes [c*npc, (c+1)*npc), split
    into blocks of BLK=111 dst nodes; each block's edge list is padded to nch
    128-edge chunks; chunks are processed in gather batches of g chunks.
  - Node table xlc[n] = [4|att|*xl(128) | xl(128)] bf16 computed on device
    (one matmul per 128-node tile), stored to DRAM, row-gathered per edge.
  - Host prebuilds lstb [128, epc] bf16 per core: rows 0:111 onehot(dstrel),
    rows 111:127 edge_attr.T, row 127 zero.  One sequential DMA per batch
    replaces onehot building on compute engines.
  - Per chunk the message (scaled, feature-major) accumulates in PSUM with
    two matmuls: mT[f,e] = xla[src].T (lhsT=gathered rows, rhs=I) +
    blk_mat.T @ lst (xr[dst] + W_e*ea, all pre-scaled by 4|att|).
  - ONE batched Prelu (leaky, alpha=0.2) over [128, g*128] computes
    l = leaky(mT) -> logits via per-chunk PE matvec e_ps[:,k] = l_k.T @ sign(att)
    (= 4*logit); pb = exp(0.25*e_ps) batched.  Segment-max subtraction is
    skipped (logits bounded, cancels in alpha = p/denom).
  - Scatter-add via onehot matmul: u_ps[111,129] += oh.T @ [pb*xl[src] | pb]
    accumulated in PSUM over the block's chunks; epilogue divides, adds bias,
    applies leaky, DMAs out.  BatchNorm on host.
"""
import sys

if "/opt/trn_rl_repo" not in sys.path:
    sys.path.insert(0, "/opt/trn_rl_repo")

import numpy as np
import ml_dtypes

BF16 = ml_dtypes.bfloat16

NEG_SLOPE = 0.2
BN_EPS = 1e-5

P = 128
NCORES = 8
BLK = 111            # dst nodes per block (111 + 16 + 1 = 128 = fused K)
F = 128              # feature dim
ED = 16              # edge-attr dim
G = 12               # chunks per gather batch (PSUM: 2*3 + 1 + 1 = 8 banks)


class Plan:
    """Geometry + host-prepped per-core inputs for one problem size."""

    def __init__(self, x, edge_attr, edge_index, W_l, W_r, W_e, att, bias,
                 ncores=NCORES):
        x = np.ascontiguousarray(np.asarray(x, dtype=np.float32))
        edge_attr = np.ascontiguousarray(np.asarray(edge_attr, dtype=np.float32))
        W_l = np.asarray(W_l, dtype=np.float32)
        W_r = np.asarray(W_r, dtype=np.float32)
        W_e = np.asarray(W_e, dtype=np.float32)
        att = np.asarray(att, dtype=np.float32)
        bias = np.asarray(bias, dtype=np.float32)
        src = np.asarray(edge_index[0]).astype(np.int64)
        dst = np.asarray(edge_index[1]).astype(np.int64)

        n = x.shape[0]
        self.n = n
        self.ncores = ncores
        self.npc = -(-n // ncores)                  # dst nodes per core
        self.nblk = -(-self.npc // BLK)             # blocks per core
        self.nt = -(-n // P)                        # transform tiles
        self.npad = self.nt * P
        need = (ncores - 1) * self.npc + self.nblk * BLK
        while self.npad < need:
            self.nt += 1
            self.npad = self.nt * P
        assert self.npad < 32768, "dma_gather int16 indices"

        order = np.argsort(dst, kind="stable")
        src_s, dst_s, ea_s = src[order], dst[order], edge_attr[order]

        blk_lo = np.empty(ncores * self.nblk, dtype=np.int64)
        blk_hi = np.empty(ncores * self.nblk, dtype=np.int64)
        for c in range(ncores):
            for j in range(self.nblk):
                i = c * self.nblk + j
                lo_node = c * self.npc + j * BLK
                hi_node = min(lo_node + BLK, (c + 1) * self.npc)
                blk_lo[i] = np.searchsorted(dst_s, lo_node)
                blk_hi[i] = np.searchsorted(dst_s, hi_node)
        counts = blk_hi - blk_lo
        nch = max(G, int(-(-counts.max() // P)))
        nch += (-nch) % G                           # multiple of G
        self.nch = nch
        self.epc = self.nblk * nch * P              # padded edges per core
        self.nchc = self.nblk * nch                 # chunks per core

        aab = (4.0 * np.abs(att)).astype(np.float32)
        self.aab = aab
        # device transform weights
        w_la = (W_l * aab[None, :]).astype(np.float32)
        self.wcat2 = np.ascontiguousarray(
            np.concatenate([w_la, W_l], axis=1)).astype(BF16)       # [F, 2F]
        self.wra = np.ascontiguousarray(
            W_r * aab[None, :]).astype(BF16)                        # [F, F]
        wea17 = np.zeros((ED + 1, F), dtype=np.float32)
        wea17[:ED] = W_e * aab[None, :]
        self.wea17 = wea17.astype(BF16)                             # [17, F]
        self.vsgn = np.ascontiguousarray(
            np.where(att >= 0, 1.0, -1.0)[:, None]).astype(BF16)    # [F, 1]
        self.ident = np.eye(P, dtype=np.float32).astype(BF16)       # [P, P]
        self.bias_bc = np.ascontiguousarray(
            np.tile(bias[None, :], (BLK, 1)))                       # [BLK, F]
        self.iota_r = np.ascontiguousarray(
            np.tile(np.arange(BLK, dtype=np.float32)[None, :], (P, 1)))

        xt = np.zeros((F, self.npad), dtype=np.float32)
        xt[:, :n] = x.T
        self.xtb = xt.astype(BF16)                                  # [F, npad]

        self.cores = []
        for c in range(ncores):
            lstb = np.zeros((P, self.epc), dtype=np.float32)
            srcidx = np.zeros(self.epc, dtype=np.int16)
            dstrel = np.full(self.epc, 120.0, dtype=np.float32)
            epos = np.zeros(self.epc, dtype=bool)
            ea_core = np.zeros((self.epc, ED), dtype=np.float32)
            dst_core = np.zeros(self.epc, dtype=np.int64)
            for j in range(self.nblk):
                i = c * self.nblk + j
                lo, hi = blk_lo[i], blk_hi[i]
                m = hi - lo
                if m == 0:
                    continue
                base = j * nch * P
                assert m <= nch * P
                sl = slice(base, base + m)
                srcidx[sl] = src_s[lo:hi]
                dstrel[sl] = dst_s[lo:hi] - c * self.npc - j * BLK
                epos[sl] = True
                ea_core[sl] = ea_s[lo:hi]
                dst_core[sl] = dst_s[lo:hi] - c * self.npc - j * BLK
            ii = np.nonzero(epos)[0]
            lstb[dst_core[ii].astype(np.int64), ii] = 1.0
            lstb[BLK:BLK + ED, :] = ea_core.T
            srcw = np.tile(srcidx.reshape(self.epc // 16, 16).T, (8, 1))
            self.cores.append(dict(
                lstb=np.ascontiguousarray(lstb.astype(BF16)),
                srcw=np.ascontiguousarray(srcw),
                dstc=np.ascontiguousarray(
                    dstrel.reshape(self.nchc, P).T),     # [P, nchc]
                xtcb=np.ascontiguousarray(
                    self.xtb[:, c * self.npc: c * self.npc
                             + self.nblk * BLK]),
            ))

    def in_maps(self):
        shared = dict(xtb=self.xtb, wcat2=self.wcat2, wra=self.wra,
                      wea17=self.wea17, vsgn=self.vsgn, ident=self.ident,
                      biasr=self.bias_bc, iotar=self.iota_r)
        return [dict(shared, **c) for c in self.cores]


def build_program(plan, num_devices=None, nch_run=None, nblk_run=None, reps=1,
                  sim_safe=False):
    import concourse.bacc as bacc
    import concourse.mybir as mybir
    import concourse.tile as tile

    dt = mybir.dt
    f32 = dt.float32
    bf16 = dt.bfloat16
    AF = mybir.ActivationFunctionType
    OP = mybir.AluOpType
    ts = lambda i, sz: slice(i * sz, (i + 1) * sz)

    nch, nblk, nt, npad = plan.nch, plan.nblk, plan.nt, plan.npad
    epc = plan.epc
    nch_run = nch if nch_run is None else nch_run      # timing experiments
    nblk_run = nblk if nblk_run is None else nblk_run
    g = G
    GROW = 256                       # gather row: [xla(128) | xl(128)] bf16

    nc = bacc.Bacc("TRN2", target_bir_lowering=False, debug=False,
                   num_devices=num_devices or plan.ncores)

    t_xtb = nc.dram_tensor("xtb", [F, npad], bf16, kind="ExternalInput")
    t_xtcb = nc.dram_tensor("xtcb", [F, nblk * BLK], bf16, kind="ExternalInput")
    t_wcat2 = nc.dram_tensor("wcat2", [F, 2 * F], bf16, kind="ExternalInput")
    t_wra = nc.dram_tensor("wra", [F, F], bf16, kind="ExternalInput")
    t_wea17 = nc.dram_tensor("wea17", [ED + 1, F], bf16, kind="ExternalInput")
    t_vsgn = nc.dram_tensor("vsgn", [F, 1], bf16, kind="ExternalInput")
    t_ident = nc.dram_tensor("ident", [P, P], bf16, kind="ExternalInput")
    t_biasr = nc.dram_tensor("biasr", [BLK, F], f32, kind="ExternalInput")
    t_iotar = nc.dram_tensor("iotar", [P, BLK], f32, kind="ExternalInput")
    t_lstb = nc.dram_tensor("lstb", [P, epc], bf16, kind="ExternalInput")
    t_srcw = nc.dram_tensor("srcw", [P, epc // 16], dt.int16,
                            kind="ExternalInput")
    t_dstc = nc.dram_tensor("dstc", [P, plan.nchc], f32, kind="ExternalInput")

    t_xlc = nc.dram_tensor("xlc", [npad, GROW], bf16, kind="Internal")
    t_out = nc.dram_tensor("out", [nblk * BLK, F], f32, kind="ExternalOutput")

    with tile.TileContext(nc) as tc:
      for _rep in range(reps):   # timing-only: amortize dispatch in slope
        with tc.tile_pool(name="resident", bufs=1) as rpool:

            # ---------- phase T: node transforms ----------
            wcat2_sb = rpool.tile([F, 2 * F], bf16, tag="wcat2")
            nc.sync.dma_start(wcat2_sb[:], t_wcat2.ap())
            wra_sb = rpool.tile([F, F], bf16, tag="wra")
            nc.sync.dma_start(wra_sb[:], t_wra.ap())
            vsgn_sb = rpool.tile([F, 1], bf16, tag="vsgn")
            nc.sync.dma_start(vsgn_sb[:], t_vsgn.ap())
            ident_sb = rpool.tile([P, P], bf16, tag="ident")
            nc.sync.dma_start(ident_sb[:], t_ident.ap())
            biasr_sb = rpool.tile([BLK, F], f32, tag="biasr")
            nc.sync.dma_start(biasr_sb[:], t_biasr.ap())
            dstc_sb = rpool.tile([P, plan.nchc], f32, tag="dstc")
            nc.sync.dma_start(dstc_sb[:], t_dstc.ap())
            srcw_sb = rpool.tile([P, epc // 16], dt.int16, tag="srcw")
            nc.sync.dma_start(srcw_sb[:], t_srcw.ap())
            iota_r = rpool.tile([P, BLK], f32, tag="iotar")
            nc.sync.dma_start(iota_r[:], t_iotar.ap())

            blk_sb = [rpool.tile([P, F], bf16, tag=f"blk{b}",
                                 name=f"blk{b}")
                      for b in range(nblk)]

            with tc.tile_pool(name="xbig", bufs=1) as xbig, \
                 tc.tile_pool(name="xstage", bufs=3) as xstg, \
                 tc.tile_pool(name="xpsum", bufs=2, space="PSUM") as xpsum:
                xtb_sb = xbig.tile([F, npad], bf16, tag="xtb")
                nc.sync.dma_start(xtb_sb[:], t_xtb.ap())
                xtcb_sb = xbig.tile([F, nblk * BLK], bf16, tag="xtcb")
                nc.sync.dma_start(xtcb_sb[:], t_xtcb.ap())
                for t in range(nt):
                    ps = xpsum.tile([P, GROW], f32, tag="xps")
                    nc.tensor.matmul(ps[:], lhsT=xtb_sb[:, ts(t, P)],
                                     rhs=wcat2_sb[:], start=True, stop=True)
                    st = xstg.tile([P, GROW], bf16, tag="xstage")
                    nc.vector.tensor_copy(st[:], ps[:])
                    nc.sync.dma_start(t_xlc.ap()[ts(t, P), :], st[:])
                for b in range(nblk):
                    ps2 = xpsum.tile([BLK, F], f32, tag="xps2")
                    nc.tensor.matmul(ps2[:], lhsT=xtcb_sb[:, ts(b, BLK)],
                                     rhs=wra_sb[:], start=True, stop=True)
                    nc.vector.tensor_copy(blk_sb[b][0:BLK, :], ps2[:])
                    nc.sync.dma_start(blk_sb[b][BLK:P, :], t_wea17.ap())

            # ---------- phase E: edges ----------
            with tc.tile_pool(name="edges", bufs=3) as epool, \
                 tc.tile_pool(name="gpool", bufs=2) as gpool, \
                 tc.tile_pool(name="small", bufs=3) as spool, \
                 tc.tile_pool(name="chunk", bufs=8) as cpool, \
                 tc.tile_pool(name="mpsum", bufs=2, space="PSUM") as mpsum, \
                 tc.tile_pool(name="epsum", bufs=1, space="PSUM") as epsum, \
                 tc.tile_pool(name="upsum", bufs=1, space="PSUM") as upsum, \
                 tc.tile_pool(name="outp", bufs=2) as opool:
                for b in range(nblk_run):
                    u_ps = upsum.tile([BLK, F + 1], f32, tag="useg")
                    # one gather per block: amortizes the slow gpsimd SWDGE
                    be0 = b * nch * P
                    xg = gpool.tile([P, nch_run, GROW], bf16, tag="xg")
                    nc.gpsimd.dma_gather(
                        xg[:], t_xlc.ap(),
                        srcw_sb[:, be0 // 16:(be0 + nch_run * P) // 16],
                        nch_run * P, nch_run * P, GROW, single_packet=False)
                    for h in range(nch_run // g):
                        q0 = b * nch + h * g
                        e0 = q0 * P
                        k0 = h * g
                        lst = epool.tile([P, g * P], bf16, tag="lst")
                        nc.sync.dma_start(lst[:], t_lstb.ap()[:, e0:e0 + g * P])
                        # one PSUM accumulation group per 2KB bank (4 chunks):
                        # 4x transpose-in (zeroing writes), then 4x msg matmul
                        # accumulates with shared blk weights; stop on last.
                        m_ps = mpsum.tile([P, g, F], f32, tag="mps")
                        for kb in range(g // 4):
                            for k in range(4 * kb, 4 * kb + 4):
                                nc.tensor.matmul(m_ps[:, k, :],
                                                 lhsT=xg[:, k0 + k, 0:F],
                                                 rhs=ident_sb[:],
                                                 start=(k % 4 == 0),
                                                 stop=False)
                            for k in range(4 * kb, 4 * kb + 4):
                                nc.tensor.matmul(m_ps[:, k, :],
                                                 lhsT=blk_sb[b][:],
                                                 rhs=lst[:, ts(k, P)],
                                                 start=False,
                                                 stop=(k % 4 == 3))
                        lk = epool.tile([P, g, P], bf16, tag="lk")
                        if sim_safe:
                            # Prelu == 0.8*relu(x) + 0.2*x, interp lacks Prelu
                            lr = epool.tile([P, g, P], f32, tag="lr")
                            nc.scalar.activation(lr[:, :, :], m_ps[:, :, :],
                                                 AF.Relu,
                                                 scale=1.0 - NEG_SLOPE)
                            lk2 = epool.tile([P, g, P], f32, tag="lk2")
                            nc.vector.tensor_scalar(
                                lk2[:, :, :], m_ps[:, :, :], NEG_SLOPE, None,
                                OP.mult)
                            nc.vector.tensor_tensor(
                                lk[:, :, :], lk2[:, :, :], lr[:, :, :], OP.add)
                        else:
                            nc.scalar.activation(lk[:, :, :], m_ps[:, :, :],
                                                 AF.Prelu, alpha=NEG_SLOPE)
                        # all g matvec columns form ONE psum group (same bank)
                        e_ps = epsum.tile([P, g], f32, tag="eps")
                        for k in range(g):
                            nc.tensor.matmul(e_ps[:, k:k + 1],
                                             lhsT=lk[:, k, :],
                                             rhs=vsgn_sb[:],
                                             start=(k == 0),
                                             stop=(k == g - 1))
                        pb = spool.tile([P, g], f32, tag="pb")
                        nc.scalar.activation(pb[:], e_ps[:], AF.Exp,
                                             scale=0.25)
                        pbb = spool.tile([P, g], bf16, tag="pbb")
                        nc.vector.tensor_copy(pbb[:], pb[:])
                        rhs2s = epool.tile([P, g, F], bf16, tag="rhs2")
                        for k in range(g):
                            q = q0 + k
                            nc.vector.tensor_scalar(
                                rhs2s[:, k, :], xg[:, k0 + k, F:2 * F],
                                pb[:, k:k + 1], None, OP.mult)
                            oh = cpool.tile([P, BLK], bf16, tag="oh")
                            nc.vector.tensor_scalar(
                                oh[:], iota_r[:], dstc_sb[:, q:q + 1], None,
                                OP.is_equal)
                            # values and denominator share one PSUM group in
                            # u_ps's bank; oh stays loaded across the pair
                            nc.tensor.matmul(
                                u_ps[:, 0:F], lhsT=oh[:], rhs=rhs2s[:, k, :],
                                start=(q == b * nch), stop=False)
                            nc.tensor.matmul(
                                u_ps[:, F:F + 1], lhsT=oh[:],
                                rhs=pbb[:, k:k + 1],
                                start=False,
                                stop=(q == b * nch + nch_run - 1))
                    # block epilogue: out = leaky(U/D + bias)
                    u_sb = opool.tile([BLK, F + 1], f32, tag="usb")
                    nc.vector.tensor_copy(u_sb[:], u_ps[:])
                    dcol = opool.tile([BLK, 1], f32, tag="dcol")
                    nc.vector.tensor_scalar(dcol[:], u_sb[:, F:F + 1], 1e-30,
                                            None, OP.add)
                    nc.vector.reciprocal(dcol[:], dcol[:])
                    ob = opool.tile([BLK, F], f32, tag="ob")
                    nc.vector.tensor_scalar(ob[:], u_sb[:, 0:F], dcol[:],
                                            None, OP.mult)
                    nc.vector.tensor_tensor(
                        ob[:], ob[:], biasr_sb[:], OP.add)
                    ob2 = opool.tile([BLK, F], f32, tag="ob2")
                    nc.vector.tensor_scalar(ob2[:], ob[:], NEG_SLOPE, None,
                                            OP.mult)
                    nc.vector.tensor_tensor(ob2[:], ob2[:], ob[:], OP.max)
                    nc.sync.dma_start(t_out.ap()[ts(b, BLK), :], ob2[:])

    nc.compile()
    return nc


def run_plan(plan, nc=None, trace=False):
    from concourse import bass_utils
    if nc is None:
        nc = build_program(plan)
    return bass_utils.run_bass_kernel_spmd(
        nc, plan.in_maps(), core_ids=list(range(plan.ncores)), trace=trace)


def assemble(plan, results):
    """Concat per-core outputs, slice to real nodes, apply host batchnorm."""
    outs = []
    for c in range(plan.ncores):
        o = np.asarray(results[c]["out"])
        lo = c * plan.npc
        take = min(plan.npc, plan.n - lo)
        outs.append(o[:take])
    out = np.concatenate(outs, axis=0)
    mean = out.mean(axis=0)
    var = out.var(axis=0)
    return ((out - mean) / np.sqrt(var + BN_EPS)).astype(np.float32)


_CACHE = {}


def kernel(x, edge_attr, edge_index, W_l, W_r, W_e, att, bias,
           bn_weight, bn_bias):
    plan = Plan(x, edge_attr, edge_index, W_l, W_r, W_e, att, bias)
    key = (plan.n, plan.nch)
    nc = _CACHE.get(key)
    if nc is None:
        nc = build_program(plan)
        _CACHE[key] = nc
    res = run_plan(plan, nc=nc)
    out = assemble(plan, res.results)
    bn_w = np.asarray(bn_weight, dtype=np.float32)
    bn_b = np.asarray(bn_bias, dtype=np.float32)
    return (out * bn_w[None, :] + bn_b[None, :]).astype(np.float32)


# revision 55
# speedup vs baseline: 234.0546x; 2.8160x over previous
"""GATv2 layer (heads=1) + post leaky-relu + batchnorm on 8 Trainium2 cores.

v2 strategy (dst-sharded edge parallelism, feature-major message pipeline):
  - Host sorts edges by dst. Core c owns dst nodes [c*npc, (c+1)*npc), split
    into blocks of BLK=111 dst nodes; each block's edge list is padded to nch
    128-edge chunks; chunks are processed in gather batches of g chunks.
  - Node table xlc[n] = [4|att|*xl(128) | xl(128)] bf16 computed on device
    (one matmul per 128-node tile), stored to DRAM, row-gathered per edge.
  - Host prebuilds lstb [128, epc] bf16 per core: rows 0:111 onehot(dstrel),
    rows 111:127 edge_attr.T, row 127 zero.  One sequential DMA per batch
    replaces onehot building on compute engines.
  - Per chunk the message (scaled, feature-major) accumulates in PSUM with
    two matmuls: mT[f,e] = xla[src].T (lhsT=gathered rows, rhs=I) +
    blk_mat.T @ lst (xr[dst] + W_e*ea, all pre-scaled by 4|att|).
  - ONE batched Prelu (leaky, alpha=0.2) over [128, g*128] computes
    l = leaky(mT) -> logits via per-chunk PE matvec e_ps[:,k] = l_k.T @ sign(att)
    (= 4*logit); pb = exp(0.25*e_ps) batched.  Segment-max subtraction is
    skipped (logits bounded, cancels in alpha = p/denom).
  - Scatter-add via onehot matmul: u_ps[111,129] += oh.T @ [pb*xl[src] | pb]
    accumulated in PSUM over the block's chunks; epilogue divides, adds bias,
    applies leaky, DMAs out.  BatchNorm on host.
"""
import sys

if "/opt/trn_rl_repo" not in sys.path:
    sys.path.insert(0, "/opt/trn_rl_repo")

import numpy as np
import ml_dtypes

BF16 = ml_dtypes.bfloat16

NEG_SLOPE = 0.2
BN_EPS = 1e-5

P = 128
NCORES = 8
BLK = 111            # dst nodes per block (111 + 16 + 1 = 128 = fused K)
F = 128              # feature dim
ED = 16              # edge-attr dim
G = 10               # chunks per compute batch (PSUM: 2*3 + 1 + 1 = 8 banks)
GS = 20              # chunks per dma_gather (2 batches)


class Plan:
    """Geometry + host-prepped per-core inputs for one problem size."""

    def __init__(self, x, edge_attr, edge_index, W_l, W_r, W_e, att, bias,
                 ncores=NCORES, sort_src=True):
        x = np.ascontiguousarray(np.asarray(x, dtype=np.float32))
        edge_attr = np.ascontiguousarray(np.asarray(edge_attr, dtype=np.float32))
        W_l = np.asarray(W_l, dtype=np.float32)
        W_r = np.asarray(W_r, dtype=np.float32)
        W_e = np.asarray(W_e, dtype=np.float32)
        att = np.asarray(att, dtype=np.float32)
        bias = np.asarray(bias, dtype=np.float32)
        src = np.asarray(edge_index[0]).astype(np.int64)
        dst = np.asarray(edge_index[1]).astype(np.int64)

        n = x.shape[0]
        self.n = n
        self.ncores = ncores
        self.npc = -(-n // ncores)                  # dst nodes per core
        self.nblk = -(-self.npc // BLK)             # blocks per core
        self.nt = -(-n // P)                        # transform tiles
        self.npad = self.nt * P
        need = (ncores - 1) * self.npc + self.nblk * BLK
        while self.npad < need:
            self.nt += 1
            self.npad = self.nt * P
        assert self.npad < 32768, "dma_gather int16 indices"

        order = np.argsort(dst, kind="stable")
        src_s, dst_s, ea_s = src[order], dst[order], edge_attr[order]
        blkid = dst_s // self.npc * self.nblk + (dst_s % self.npc) // BLK
        if sort_src:
            # within each dst-block the edge order is free: sort by src so
            # the per-edge row gather reads HBM near-sequentially
            order2 = np.lexsort((src_s, blkid))
            src_s, dst_s, ea_s = src_s[order2], dst_s[order2], ea_s[order2]
            blkid = blkid[order2]

        nb = ncores * self.nblk
        blk_lo = np.searchsorted(blkid, np.arange(nb), side="left")
        blk_hi = np.searchsorted(blkid, np.arange(nb), side="right")
        counts = blk_hi - blk_lo
        nch = max(GS, int(-(-counts.max() // P)))
        nch += (-nch) % GS                          # multiple of GS
        self.nch = nch
        self.epc = self.nblk * nch * P              # padded edges per core
        self.nchc = self.nblk * nch                 # chunks per core

        aab = (4.0 * np.abs(att)).astype(np.float32)
        self.aab = aab
        # device transform weights
        w_la = (W_l * aab[None, :]).astype(np.float32)
        self.wcat2 = np.ascontiguousarray(
            np.concatenate([w_la, W_l], axis=1)).astype(BF16)       # [F, 2F]
        self.wra = np.ascontiguousarray(
            W_r * aab[None, :]).astype(BF16)                        # [F, F]
        wea17 = np.zeros((ED + 1, F), dtype=np.float32)
        wea17[:ED] = W_e * aab[None, :]
        self.wea17 = wea17.astype(BF16)                             # [17, F]
        self.vsgn = np.ascontiguousarray(
            np.where(att >= 0, 1.0, -1.0)[:, None]).astype(BF16)    # [F, 1]
        self.ident = np.eye(P, dtype=np.float32).astype(BF16)       # [P, P]
        self.bias_bc = np.ascontiguousarray(
            np.tile(bias[None, :], (BLK, 1)))                       # [BLK, F]
        self.invsc = np.ascontiguousarray(
            np.tile((1.0 / aab)[None, :], (BLK, 1)))                # [BLK, F]
        self.iota_r = np.ascontiguousarray(
            np.tile(np.arange(BLK, dtype=np.float32)[None, :], (P, 1)))

        xt = np.zeros((F, self.npad), dtype=np.float32)
        xt[:, :n] = x.T
        self.xtb = xt.astype(BF16)                                  # [F, npad]

        self.cores = []
        for c in range(ncores):
            lstb = np.zeros((P, self.epc), dtype=np.float32)
            # pad slots get idx -1: trailing negatives produce NO gather
            # descriptors (skipped), cutting ~13% of gather work
            srcidx = np.full(self.epc, -1, dtype=np.int16)
            dstrel = np.full(self.epc, 120.0, dtype=np.float32)
            epos = np.zeros(self.epc, dtype=bool)
            ea_core = np.zeros((self.epc, ED), dtype=np.float32)
            dst_core = np.zeros(self.epc, dtype=np.int64)
            for j in range(self.nblk):
                i = c * self.nblk + j
                lo, hi = blk_lo[i], blk_hi[i]
                m = hi - lo
                if m == 0:
                    continue
                base = j * nch * P
                assert m <= nch * P
                sl = slice(base, base + m)
                srcidx[sl] = src_s[lo:hi]
                dstrel[sl] = dst_s[lo:hi] - c * self.npc - j * BLK
                epos[sl] = True
                ea_core[sl] = ea_s[lo:hi]
                dst_core[sl] = dst_s[lo:hi] - c * self.npc - j * BLK
            ii = np.nonzero(epos)[0]
            lstb[dst_core[ii].astype(np.int64), ii] = 1.0
            lstb[BLK:BLK + ED, :] = ea_core.T
            srcw = np.tile(srcidx.reshape(self.epc // 16, 16).T, (8, 1))
            nspan = self.epc // (GS * P)
            gcnt = (srcidx.reshape(nspan, GS * P) >= 0).sum(1)
            self.cores.append(dict(
                lstb=np.ascontiguousarray(lstb.astype(BF16)),
                srcw=np.ascontiguousarray(srcw),
                gcnt=np.ascontiguousarray(
                    gcnt[None, :].astype(np.int32)),     # [1, nspan]
                dstc=np.ascontiguousarray(
                    dstrel.reshape(self.nchc, P).T),     # [P, nchc]
                xtcb=np.ascontiguousarray(
                    self.xtb[:, c * self.npc: c * self.npc
                             + self.nblk * BLK]),
            ))

    def in_maps(self):
        shared = dict(xtb=self.xtb, wcat2=self.wcat2, wra=self.wra,
                      wea17=self.wea17, vsgn=self.vsgn, ident=self.ident,
                      biasr=self.bias_bc, iotar=self.iota_r,
                      invsc=self.invsc)
        return [dict(shared, **c) for c in self.cores]


def build_program(plan, num_devices=None, nch_run=None, nblk_run=None, reps=1,
                  sim_safe=False, row256=False, skip_gather=False, gs=None,
                  gbufs=8):
    import concourse.bacc as bacc
    import concourse.mybir as mybir
    import concourse.tile as tile

    dt = mybir.dt
    f32 = dt.float32
    bf16 = dt.bfloat16
    AF = mybir.ActivationFunctionType
    OP = mybir.AluOpType
    ts = lambda i, sz: slice(i * sz, (i + 1) * sz)

    nch, nblk, nt, npad = plan.nch, plan.nblk, plan.nt, plan.npad
    epc = plan.epc
    nch_run = nch if nch_run is None else nch_run      # timing experiments
    nblk_run = nblk if nblk_run is None else nblk_run
    g, gs = G, (gs or GS)
    assert gs == GS, "gcnt spans are GS-sized"
    GROW = 128 if row256 else 256    # gather row [xla] or [xla | xl] bf16

    NQ = 4
    nc = bacc.Bacc("TRN2", target_bir_lowering=False, debug=False,
                   num_devices=num_devices or plan.ncores,
                   num_swdge_queues=NQ)

    t_xtb = nc.dram_tensor("xtb", [F, npad], bf16, kind="ExternalInput")
    t_xtcb = nc.dram_tensor("xtcb", [F, nblk * BLK], bf16, kind="ExternalInput")
    t_wcat2 = nc.dram_tensor("wcat2", [F, 2 * F], bf16, kind="ExternalInput")
    t_wra = nc.dram_tensor("wra", [F, F], bf16, kind="ExternalInput")
    t_wea17 = nc.dram_tensor("wea17", [ED + 1, F], bf16, kind="ExternalInput")
    t_vsgn = nc.dram_tensor("vsgn", [F, 1], bf16, kind="ExternalInput")
    t_ident = nc.dram_tensor("ident", [P, P], bf16, kind="ExternalInput")
    t_biasr = nc.dram_tensor("biasr", [BLK, F], f32, kind="ExternalInput")
    t_iotar = nc.dram_tensor("iotar", [P, BLK], f32, kind="ExternalInput")
    t_invsc = nc.dram_tensor("invsc", [BLK, F], f32, kind="ExternalInput")
    t_lstb = nc.dram_tensor("lstb", [P, epc], bf16, kind="ExternalInput")
    t_srcw = nc.dram_tensor("srcw", [P, epc // 16], dt.int16,
                            kind="ExternalInput")
    t_gcnt = nc.dram_tensor("gcnt", [1, epc // (GS * P)], dt.int32,
                            kind="ExternalInput")
    t_dstc = nc.dram_tensor("dstc", [P, plan.nchc], f32, kind="ExternalInput")

    t_xlc = nc.dram_tensor("xlc", [npad, GROW], bf16, kind="Internal")
    t_out = nc.dram_tensor("out", [nblk * BLK, F], f32, kind="ExternalOutput")

    with tile.TileContext(nc) as tc:
      for _rep in range(reps):   # timing-only: amortize dispatch in slope
        with tc.tile_pool(name="resident", bufs=1) as rpool:

            # ---------- phase T: node transforms ----------
            wcat2_sb = rpool.tile([F, 2 * F], bf16, tag="wcat2")
            nc.sync.dma_start(wcat2_sb[:], t_wcat2.ap())
            wra_sb = rpool.tile([F, F], bf16, tag="wra")
            nc.sync.dma_start(wra_sb[:], t_wra.ap())
            vsgn_sb = rpool.tile([F, 1], bf16, tag="vsgn")
            nc.sync.dma_start(vsgn_sb[:], t_vsgn.ap())
            ident_sb = rpool.tile([P, P], bf16, tag="ident")
            nc.sync.dma_start(ident_sb[:], t_ident.ap())
            biasr_sb = rpool.tile([BLK, F], f32, tag="biasr")
            nc.sync.dma_start(biasr_sb[:], t_biasr.ap())
            if row256:
                invsc_sb = rpool.tile([BLK, F], f32, tag="invsc")
                nc.sync.dma_start(invsc_sb[:], t_invsc.ap())
            dstc_sb = rpool.tile([P, plan.nchc], f32, tag="dstc")
            nc.sync.dma_start(dstc_sb[:], t_dstc.ap())
            srcw_sb = rpool.tile([P, epc // 16], dt.int16, tag="srcw")
            nc.sync.dma_start(srcw_sb[:], t_srcw.ap())
            gcnt_sb = rpool.tile([1, epc // (GS * P)], dt.int32, tag="gcnt")
            nc.sync.dma_start(gcnt_sb[:], t_gcnt.ap())
            greg = nc.gpsimd.alloc_register(f"gcnt_reg{_rep}")
            iota_r = rpool.tile([P, BLK], f32, tag="iotar")
            nc.sync.dma_start(iota_r[:], t_iotar.ap())

            blk_sb = [rpool.tile([P, F], bf16, tag=f"blk{b}",
                                 name=f"blk{b}")
                      for b in range(nblk)]

            with tc.tile_pool(name="xbig", bufs=1) as xbig, \
                 tc.tile_pool(name="xstage", bufs=3) as xstg, \
                 tc.tile_pool(name="xpsum", bufs=2, space="PSUM") as xpsum:
                xtb_sb = xbig.tile([F, npad], bf16, tag="xtb")
                nc.sync.dma_start(xtb_sb[:], t_xtb.ap())
                xtcb_sb = xbig.tile([F, nblk * BLK], bf16, tag="xtcb")
                nc.sync.dma_start(xtcb_sb[:], t_xtcb.ap())
                for t in range(nt):
                    ps = xpsum.tile([P, GROW], f32, tag="xps")
                    nc.tensor.matmul(ps[:], lhsT=xtb_sb[:, ts(t, P)],
                                     rhs=wcat2_sb[:, 0:GROW],
                                     start=True, stop=True)
                    st = xstg.tile([P, GROW], bf16, tag="xstage")
                    nc.vector.tensor_copy(st[:], ps[:])
                    nc.sync.dma_start(t_xlc.ap()[ts(t, P), :], st[:])
                for b in range(nblk):
                    ps2 = xpsum.tile([BLK, F], f32, tag="xps2")
                    nc.tensor.matmul(ps2[:], lhsT=xtcb_sb[:, ts(b, BLK)],
                                     rhs=wra_sb[:], start=True, stop=True)
                    nc.vector.tensor_copy(blk_sb[b][0:BLK, :], ps2[:])
                    nc.sync.dma_start(blk_sb[b][BLK:P, :], t_wea17.ap())

            # ---------- phase E: edges ----------
            with tc.tile_pool(name="edges", bufs=3) as epool, \
                 tc.tile_pool(name="gpool", bufs=gbufs) as gpool, \
                 tc.tile_pool(name="small", bufs=3) as spool, \
                 tc.tile_pool(name="chunk", bufs=8) as cpool, \
                 tc.tile_pool(name="mpsum", bufs=2, space="PSUM") as mpsum, \
                 tc.tile_pool(name="epsum", bufs=1, space="PSUM") as epsum, \
                 tc.tile_pool(name="upsum", bufs=1, space="PSUM") as upsum, \
                 tc.tile_pool(name="outp", bufs=2) as opool:
                gcount = 0
                for b in range(nblk_run):
                    u_ps = upsum.tile([BLK, F + 1], f32, tag="useg")
                    for hh in range(nch_run // g):
                        # gs-chunk gathers amortize the slow gpsimd SWDGE;
                        # rotate SWDGE queues so descriptor processing of
                        # consecutive gathers runs in parallel
                        if (hh * g) % gs == 0:
                            ge0 = (b * nch + hh * g) * P
                            xg = gpool.tile([P, gs, GROW], bf16, tag="xg")
                            if gcount < gbufs:
                                # first use of each rotating buffer: clear so
                                # skipped pad slots read finite values
                                nc.vector.memset(xg[:], 0.0)
                            nidx = P if skip_gather else gs * P
                            span = ge0 // (GS * P)
                            nc.gpsimd.reg_load(
                                greg, gcnt_sb[0:1, span:span + 1])
                            nc.gpsimd.dma_gather(
                                xg[:, 0:nidx // P, :], t_xlc.ap(),
                                srcw_sb[:, ge0 // 16:(ge0 + nidx) // 16],
                                nidx, nidx if skip_gather else greg,
                                GROW, single_packet=False,
                                queue_num=gcount % NQ)
                            gcount += 1
                        q0 = b * nch + hh * g
                        e0 = q0 * P
                        k0 = (hh * g) % gs
                        lst = epool.tile([P, g * P], bf16, tag="lst")
                        nc.sync.dma_start(lst[:], t_lstb.ap()[:, e0:e0 + g * P])
                        # one PSUM accumulation group per 2KB bank (4 chunks):
                        # 4x transpose-in (zeroing writes), then 4x msg matmul
                        # accumulates with shared blk weights; stop on last.
                        m_ps = mpsum.tile([P, g, F], f32, tag="mps")
                        for k4 in range(0, g, 4):
                            kq = list(range(k4, min(k4 + 4, g)))
                            for k in kq:
                                nc.tensor.matmul(m_ps[:, k, :],
                                                 lhsT=xg[:, k0 + k, 0:F],
                                                 rhs=ident_sb[:],
                                                 start=(k == kq[0]),
                                                 stop=False)
                            for k in kq:
                                nc.tensor.matmul(m_ps[:, k, :],
                                                 lhsT=blk_sb[b][:],
                                                 rhs=lst[:, ts(k, P)],
                                                 start=False,
                                                 stop=(k == kq[-1]))
                        lk = epool.tile([P, g, P], bf16, tag="lk")
                        if sim_safe:
                            # Prelu == 0.8*relu(x) + 0.2*x, interp lacks Prelu
                            lr = epool.tile([P, g, P], f32, tag="lr")
                            nc.scalar.activation(lr[:, :, :], m_ps[:, :, :],
                                                 AF.Relu,
                                                 scale=1.0 - NEG_SLOPE)
                            lk2 = epool.tile([P, g, P], f32, tag="lk2")
                            nc.vector.tensor_scalar(
                                lk2[:, :, :], m_ps[:, :, :], NEG_SLOPE, None,
                                OP.mult)
                            nc.vector.tensor_tensor(
                                lk[:, :, :], lk2[:, :, :], lr[:, :, :], OP.add)
                        else:
                            nc.scalar.activation(lk[:, :, :], m_ps[:, :, :],
                                                 AF.Prelu, alpha=NEG_SLOPE)
                        # all g matvec columns form ONE psum group (same bank)
                        e_ps = epsum.tile([P, g], f32, tag="eps")
                        for k in range(g):
                            nc.tensor.matmul(e_ps[:, k:k + 1],
                                             lhsT=lk[:, k, :],
                                             rhs=vsgn_sb[:],
                                             start=(k == 0),
                                             stop=(k == g - 1))
                        pb = spool.tile([P, g], f32, tag="pb")
                        nc.scalar.activation(pb[:], e_ps[:], AF.Exp,
                                             scale=0.25)
                        pbb = spool.tile([P, g], bf16, tag="pbb")
                        nc.vector.tensor_copy(pbb[:], pb[:])
                        rhs2s = epool.tile([P, g, F], bf16, tag="rhs2")
                        vlo = 0 if row256 else F
                        for k in range(g):
                            q = q0 + k
                            nc.vector.tensor_scalar(
                                rhs2s[:, k, :], xg[:, k0 + k, vlo:vlo + F],
                                pb[:, k:k + 1], None, OP.mult)
                            oh = cpool.tile([P, BLK], bf16, tag="oh")
                            nc.vector.tensor_scalar(
                                oh[:], iota_r[:], dstc_sb[:, q:q + 1], None,
                                OP.is_equal)
                            # values and denominator share one PSUM group in
                            # u_ps's bank; oh stays loaded across the pair
                            nc.tensor.matmul(
                                u_ps[:, 0:F], lhsT=oh[:], rhs=rhs2s[:, k, :],
                                start=(q == b * nch), stop=False)
                            nc.tensor.matmul(
                                u_ps[:, F:F + 1], lhsT=oh[:],
                                rhs=pbb[:, k:k + 1],
                                start=False,
                                stop=(q == b * nch + nch_run - 1))
                    # block epilogue: out = leaky(U/D + bias)
                    u_sb = opool.tile([BLK, F + 1], f32, tag="usb")
                    nc.vector.tensor_copy(u_sb[:], u_ps[:])
                    dcol = opool.tile([BLK, 1], f32, tag="dcol")
                    nc.vector.tensor_scalar(dcol[:], u_sb[:, F:F + 1], 1e-30,
                                            None, OP.add)
                    nc.vector.reciprocal(dcol[:], dcol[:])
                    ob = opool.tile([BLK, F], f32, tag="ob")
                    nc.vector.tensor_scalar(ob[:], u_sb[:, 0:F], dcol[:],
                                            None, OP.mult)
                    if row256:   # values were 4|att|-scaled; undo per feature
                        nc.vector.tensor_tensor(
                            ob[:], ob[:], invsc_sb[:], OP.mult)
                    nc.vector.tensor_tensor(
                        ob[:], ob[:], biasr_sb[:], OP.add)
                    ob2 = opool.tile([BLK, F], f32, tag="ob2")
                    nc.vector.tensor_scalar(ob2[:], ob[:], NEG_SLOPE, None,
                                            OP.mult)
                    nc.vector.tensor_tensor(ob2[:], ob2[:], ob[:], OP.max)
                    nc.sync.dma_start(t_out.ap()[ts(b, BLK), :], ob2[:])

    nc.compile()
    return nc


def run_plan(plan, nc=None, trace=False):
    from concourse import bass_utils
    if nc is None:
        nc = build_program(plan)
    return bass_utils.run_bass_kernel_spmd(
        nc, plan.in_maps(), core_ids=list(range(plan.ncores)), trace=trace)


def assemble(plan, results):
    """Concat per-core outputs, slice to real nodes, apply host batchnorm."""
    outs = []
    for c in range(plan.ncores):
        o = np.asarray(results[c]["out"])
        lo = c * plan.npc
        take = min(plan.npc, plan.n - lo)
        outs.append(o[:take])
    out = np.concatenate(outs, axis=0)
    mean = out.mean(axis=0)
    var = out.var(axis=0)
    return ((out - mean) / np.sqrt(var + BN_EPS)).astype(np.float32)


_CACHE = {}


def kernel(x, edge_attr, edge_index, W_l, W_r, W_e, att, bias,
           bn_weight, bn_bias):
    plan = Plan(x, edge_attr, edge_index, W_l, W_r, W_e, att, bias)
    key = (plan.n, plan.nch)
    nc = _CACHE.get(key)
    if nc is None:
        nc = build_program(plan)
        _CACHE[key] = nc
    res = run_plan(plan, nc=nc)
    out = assemble(plan, res.results)
    bn_w = np.asarray(bn_weight, dtype=np.float32)
    bn_b = np.asarray(bn_bias, dtype=np.float32)
    return (out * bn_w[None, :] + bn_b[None, :]).astype(np.float32)
